# revision 16
# baseline (speedup 1.0000x reference)
"""Causal MHA (B=2, N=2048, D=1024, H=16) on 8 NeuronCores via Bass/Tile.

Sharding: core c = (b, g): b = c // 4 (batch), g = c % 4 (head group of 4
heads = 256 features). Each core computes its Q/K/V projections, causal
attention for its 4 heads, and a partial output projection (its 256 rows of
Wo). The host sums the 4 partials per batch ("unshard" of row-parallel TP).

Layout: activations are feature-major (features on SBUF partitions, sequence
on the free axis), so S^T = K Q^T tiles come out of the PE with k on
partitions and q free and exp() needs no reduction at all. The softmax
denominator falls out of the P@V matmul via a ones column appended to V; the
per-(head, q) normalization uses a reciprocal row broadcast across partitions
through a DRAM bounce.

v2 (throughput): all matmul operands are bf16 (host converts; 1 cycle/row at
any free size, vs f32r needing free>=256), which also halves DMA. Diagonal
causal blocks are trimmed to their unmasked q-range in S, exp and PV. During
attention the Activation engine runs ONLY the exp (projection epilogues and
masking/normalize run on the otherwise-idle Pool engine) since exp is the
pacing engine. PV lags S by one kt step, and ACT-independent PE work (the V
second-half pass, the previous slice's O-projection) is interleaved into the
attention loop so the PE never idles waiting for exp and keeps its 2.4 GHz
pstate (it drops to 1.2 GHz after idle gaps).
"""

import numpy as np
import ml_dtypes

import concourse.bass as bass
import concourse.bacc as bacc
import concourse.mybir as mybir
from concourse.tile import TileContext
from concourse.bass_utils import run_bass_kernel_spmd

F32 = mybir.dt.float32
BF16 = mybir.dt.bfloat16
AF = mybir.ActivationFunctionType

B, N, D, H, DH = 2, 2048, 1024, 16, 64
NCORES = 8
GROUPS = 4
HPC = H // GROUPS     # 4 heads per core
FS = HPC * DH         # 256
P = 128
NDT = N // 128        # 16
NSS = N // 512        # 4
DT = D // 128         # 8
FT = FS // 128        # 2
DH2 = DT // 2         # d-tiles per half

_CACHE = {}


def _build(repeat=1):
    nc = bacc.Bacc("TRN2", target_bir_lowering=False, debug=False)

    xqT = nc.dram_tensor("xqT", [D, N], BF16, kind="ExternalInput")
    xkvT = nc.dram_tensor("xkvT", [D, N], BF16, kind="ExternalInput")
    wq = nc.dram_tensor("wq", [D, FS], BF16, kind="ExternalInput")
    wk = nc.dram_tensor("wk", [D, FS], BF16, kind="ExternalInput")
    wv = nc.dram_tensor("wv", [D, FS], BF16, kind="ExternalInput")
    wo = nc.dram_tensor("wo", [FS, D], BF16, kind="ExternalInput")
    bq = nc.dram_tensor("bq", [FS], F32, kind="ExternalInput")
    bk = nc.dram_tensor("bk", [FS], F32, kind="ExternalInput")
    bv = nc.dram_tensor("bv", [1, FS], F32, kind="ExternalInput")
    bo = nc.dram_tensor("bo", [1, D], F32, kind="ExternalInput")
    masks = nc.dram_tensor("masks", [P, P], BF16, kind="ExternalInput")
    out = nc.dram_tensor("out_p", [N, D], BF16, kind="ExternalOutput")

    def bcast_rows(dram_ap, rows):
        # stride-0 partition AP: read DRAM row [1, F] as [rows, F]
        return bass.AP(
            tensor=dram_ap.tensor,
            offset=dram_ap.offset,
            ap=[[0, rows]] + [list(x) for x in dram_ap.ap[1:]],
        )

    with TileContext(nc) as tc:
        with (
            tc.tile_pool(name="const", bufs=1) as cp,
            tc.tile_pool(name="xt", bufs=1) as xp,
            tc.tile_pool(name="acts", bufs=1) as ap_,
            tc.tile_pool(name="ps", bufs=3, space="PSUM") as psp,
            tc.tile_pool(name="pt", bufs=3) as ptp,
            tc.tile_pool(name="small", bufs=4) as smp,
            tc.tile_pool(name="osb", bufs=3) as osp,
            tc.tile_pool(name="dsc", bufs=4, space="DRAM") as dsp,
        ):
            wo_sb = cp.tile([P, FT, D], BF16, tag="wo")
            bqk_sb = cp.tile([P, 2, 2], F32, tag="bqk")
            tri2_sb = cp.tile([P, 2, P], BF16, tag="mask")
            ones_f = cp.tile([P, HPC], BF16, tag="ones_f")
            bo_rep = cp.tile([P, D], F32, tag="bo_rep")
            bv_rep = cp.tile([P, FS], F32, tag="bv_rep")

            nc.sync.dma_start(out=wo_sb, in_=wo.ap().rearrange("(t p) f -> p t f", p=P))
            nc.sync.dma_start(out=bqk_sb[:, 0, :], in_=bk.ap().rearrange("(t p) -> p t", p=P))
            nc.sync.dma_start(out=bqk_sb[:, 1, :], in_=bq.ap().rearrange("(t p) -> p t", p=P))
            nc.sync.dma_start(out=tri2_sb[:, 0, :], in_=masks.ap())
            nc.sync.dma_start(out=tri2_sb[:, 1, :], in_=masks.ap())
            nc.sync.dma_start(out=bo_rep, in_=bcast_rows(bo.ap(), P))
            nc.sync.dma_start(out=bv_rep, in_=bcast_rows(bv.ap(), P))
            nc.vector.memset(ones_f, 1.0)

            kt_all = [ap_.tile([P, N], BF16, tag=f"kt{f}", name=f"kt{f}") for f in range(FT)]
            qt_all = [ap_.tile([P, N], BF16, tag=f"qt{f}", name=f"qt{f}") for f in range(FT)]
            v_sb = [ap_.tile([P, HPC, DH + 1], BF16, tag=f"v{st}", name=f"v{st}") for st in range(NDT)]
            ot_all = [ap_.tile([P, N], BF16, tag=f"ot{f}", name=f"ot{f}") for f in range(FT)]
            bv_rep_v = bv_rep.rearrange("p (h c) -> p h c", h=HPC)

            def emit_body():
                # ---- projections, streamed in two d-halves ----
                halves = []
                for half in range(2):
                    d0 = half * DH2
                    # weights go out on the ACT hwdge queue so the x tiles
                    # (SP queue) and the first K matmul aren't delayed
                    wk_sb = cp.tile([P, DH2, FS], BF16, tag="w", bufs=3, name="wk_h")
                    nc.scalar.dma_start(out=wk_sb, in_=wk.ap().rearrange("(t p) f -> p t f", p=P)[:, d0:d0 + DH2, :])
                    wv_sb = cp.tile([P, DH2, FS], BF16, tag="w", bufs=3, name="wv_h")
                    nc.scalar.dma_start(out=wv_sb, in_=wv.ap().rearrange("(t p) f -> p t f", p=P)[:, d0:d0 + DH2, :])
                    wq_sb = cp.tile([P, DH2, FS], BF16, tag="w", bufs=3, name="wq_h")
                    nc.scalar.dma_start(out=wq_sb, in_=wq.ap().rearrange("(t p) f -> p t f", p=P)[:, d0:d0 + DH2, :])
                    xkv_t, xq_t = [], []
                    for i in range(DH2):
                        d = d0 + i
                        t = xp.tile([P, N], BF16, tag=f"xkv{i}", name=f"xkv{i}")
                        nc.sync.dma_start(out=t, in_=xkvT.ap()[d * P:(d + 1) * P, :])
                        xkv_t.append(t)
                    for i in range(DH2):
                        d = d0 + i
                        t = xp.tile([P, N], BF16, tag=f"xq{i}", name=f"xq{i}")
                        nc.sync.dma_start(out=t, in_=xqT.ap()[d * P:(d + 1) * P, :])
                        xq_t.append(t)
                    halves.append((wk_sb, wv_sb, wq_sb, xkv_t, xq_t))

                def emit_kq(half, which, ft, ss):
                    wk_sb, wv_sb, wq_sb, xkv_t, xq_t = halves[half]
                    w_sb, x_t = (wk_sb, xkv_t) if which == 0 else (wq_sb, xq_t)
                    dst_all = kt_all if which == 0 else qt_all
                    ps = psp.tile([P, 512], F32, tag="ps", name="ps_kq")
                    for i in range(DH2):
                        nc.tensor.matmul(
                            ps,
                            w_sb[:, i, ft * P:(ft + 1) * P],
                            x_t[i][:, ss * 512:(ss + 1) * 512],
                            start=(i == 0),
                            stop=(i == DH2 - 1),
                        )
                    dst = dst_all[ft][:, ss * 512:(ss + 1) * 512]
                    if half == 0:
                        nc.vector.tensor_scalar_add(dst, ps, bqk_sb[:, which, ft:ft + 1])
                    else:
                        nc.vector.tensor_add(dst, dst, ps)

                def emit_v(half, st, pstag="ps"):
                    wk_sb, wv_sb, wq_sb, xkv_t, xq_t = halves[half]
                    psv = psp.tile([P, 512], F32, tag=pstag, bufs=(3 if pstag == "ps" else 1), name="ps_v")
                    for i in range(DH2):
                        nc.tensor.matmul(
                            psv[:, 0:FS],
                            xkv_t[i][:, st * P:(st + 1) * P],
                            wv_sb[:, i, :],
                            start=(i == 0),
                            stop=(i == DH2 - 1),
                        )
                    vdst = v_sb[st][:, :, 0:DH]
                    psv_v = psv[:, 0:FS].rearrange("p (h c) -> p h c", h=HPC)
                    if half == 0:
                        nc.vector.tensor_add(vdst, psv_v, bv_rep_v)
                        nc.gpsimd.tensor_copy(v_sb[st][:, :, DH], ones_f)
                    else:
                        nc.vector.tensor_add(vdst, vdst, psv_v)

                # half 0: K, V, Q fully
                for ft in range(FT):
                    for ss in range(NSS):
                        emit_kq(0, 0, ft, ss)
                for st in range(NDT):
                    emit_v(0, st)
                for ft in range(FT):
                    for ss in range(NSS):
                        emit_kq(0, 1, ft, ss)
                # half 1: K, Q now; V interleaved into ss=0 attention below
                for ft in range(FT):
                    for ss in range(NSS):
                        emit_kq(1, 0, ft, ss)
                for ft in range(FT):
                    for ss in range(NSS):
                        emit_kq(1, 1, ft, ss)

                def oproj_units(ss_, tail=False):
                    for qt in range(4 * ss_, 4 * ss_ + 4):
                        o_sb = osp.tile([P, D], BF16, tag="osb", name="o_sb")
                        if tail:
                            # attention is done: st2's ps2 banks are free and
                            # double-buffered, so the tail pipeline never
                            # serializes on a single PSUM bank
                            def unit(qt=qt, o_sb=o_sb):
                                pst = psp.tile([P, 1024], F32, tag="ps2", bufs=2, name="ps_o2")
                                for os_ in range(2):
                                    for ft2 in range(FT):
                                        nc.tensor.matmul(
                                            pst[:, os_ * 512:(os_ + 1) * 512],
                                            ot_all[ft2][:, qt * P:(qt + 1) * P],
                                            wo_sb[:, ft2, os_ * 512:(os_ + 1) * 512],
                                            start=(ft2 == 0),
                                            stop=(ft2 == FT - 1),
                                        )
                                nc.vector.tensor_add(o_sb, pst, bo_rep)
                                nc.sync.dma_start(out=out.ap()[qt * P:(qt + 1) * P, :], in_=o_sb)
                            yield unit
                            continue
                        for os_ in range(2):
                            def unit(qt=qt, os_=os_, o_sb=o_sb):
                                ps_o = psp.tile([P, 512], F32, tag="pso", bufs=1, name="ps_o")
                                for ft2 in range(FT):
                                    nc.tensor.matmul(
                                        ps_o,
                                        ot_all[ft2][:, qt * P:(qt + 1) * P],
                                        wo_sb[:, ft2, os_ * 512:(os_ + 1) * 512],
                                        start=(ft2 == 0),
                                        stop=(ft2 == FT - 1),
                                    )
                                nc.vector.tensor_add(
                                    o_sb[:, os_ * 512:(os_ + 1) * 512],
                                    ps_o,
                                    bo_rep[:, os_ * 512:(os_ + 1) * 512],
                                )
                                if os_ == 1:
                                    nc.sync.dma_start(out=out.ap()[qt * P:(qt + 1) * P, :], in_=o_sb)
                            yield unit

                # ---- attention: per (ss, ft), kt loop with lag-1 PV and
                # PE filler units pulled between S and PV ----
                def attention_block(ss, ft, filler, quota):
                    # quota: how many filler units to pull across this block
                    n_kt = 4 * ss + 4
                    otp = [
                        psp.tile([P, 512], F32, tag="ps", name=f"ps_ot{hh}")
                        for hh in range(2)
                    ]
                    pulled = 0
                    pending = []  # PV lags S by 2 kt steps

                    def emit_pv(kt, ptt, dk):
                        for hh in range(2):
                            nc.tensor.matmul(
                                otp[hh][0:DH + 1, dk:512],
                                v_sb[kt][:, ft * 2 + hh, :],
                                ptt[:, hh * 512 + dk:(hh + 1) * 512],
                                start=(kt == 0),
                                stop=(kt == n_kt - 1),
                            )

                    for kt in range(n_kt):
                        dk = max(0, (kt - 4 * ss) * P)
                        st2 = psp.tile([P, 1024], F32, tag="ps2", bufs=2, name="ps_st2")
                        ptt = ptp.tile([P, 1024], BF16, tag="pt", name="ptt")
                        for hh in range(2):
                            nc.tensor.matmul(
                                st2[:, hh * 512 + dk:(hh + 1) * 512],
                                kt_all[ft][hh * 64:(hh + 1) * 64, kt * P:(kt + 1) * P],
                                qt_all[ft][hh * 64:(hh + 1) * 64, ss * 512 + dk:(ss + 1) * 512],
                                start=True, stop=True,
                            )
                        if len(pending) >= 2:
                            emit_pv(*pending.pop(0))
                        want = ((kt + 1) * quota) // n_kt
                        while pulled < want:
                            u = next(filler, None)
                            pulled += 1
                            if u is None:
                                break
                            u()
                        st2v = st2.rearrange("p (h q) -> p h q", h=2)
                        pttv = ptt.rearrange("p (h q) -> p h q", h=2)
                        nc.scalar.activation(pttv[:, :, dk:512], st2v[:, :, dk:512], AF.Exp, scale=0.125)
                        if kt >= 4 * ss:  # diagonal block: mask 128 cols past dk
                            nc.gpsimd.tensor_mul(
                                pttv[:, :, dk:dk + P],
                                pttv[:, :, dk:dk + P],
                                tri2_sb,
                            )
                        pending.append((kt, ptt, dk))
                    for pv in pending:
                        emit_pv(*pv)

                    # normalization: reciprocal row, broadcast across
                    # partitions via a DRAM bounce (partition_broadcast ucode
                    # is broken in this environment)
                    rept = smp.tile([DH + 1, 1024], F32, tag="rep", bufs=2, name="rept")
                    recip = rept[DH:DH + 1, :]
                    with nc.allow_low_precision(reason="softmax reciprocal"):
                        nc.vector.reciprocal(recip[:, 0:512], otp[0][DH:DH + 1, :])
                        nc.vector.reciprocal(recip[:, 512:1024], otp[1][DH:DH + 1, :])
                    dscr = dsp.tile([1, 1024], F32, tag="dscr", name="dscr")
                    nc.sync.dma_start(out=dscr, in_=recip)
                    nc.sync.dma_start(out=rept[0:DH, :], in_=bcast_rows(dscr, DH))
                    for hh in range(2):
                        nc.vector.tensor_mul(
                            ot_all[ft][hh * 64:hh * 64 + DH, ss * 512:(ss + 1) * 512],
                            otp[hh][0:DH, :],
                            rept[0:DH, hh * 512:(hh + 1) * 512],
                        )

                # ss=0: fill with the 16 V-half1 tiles (uses the "pso" bank)
                v1 = iter([lambda st=st: emit_v(1, st, pstag="pso") for st in range(NDT)])
                attention_block(0, 0, v1, 8)
                attention_block(0, 1, v1, 8)
                # ss>=1: fill with the previous slice's O-projection
                for ss in range(1, NSS):
                    op = oproj_units(ss - 1)
                    attention_block(ss, 0, op, 4)
                    attention_block(ss, 1, op, 4)
                    for u in op:
                        u()
                for u in oproj_units(NSS - 1, tail=True):
                    u()

            if repeat == 1:
                emit_body()
            else:
                with tc.For_i(0, repeat, 1):
                    emit_body()

    nc.compile()
    return nc


def _shard_inputs(x_q, x_kv, Wq, bq_, Wk, bk_, Wv, bv_, Wo, bo_):
    pp_, ff = np.meshgrid(np.arange(P), np.arange(P), indexing="ij")
    mask = (ff >= pp_).astype(ml_dtypes.bfloat16)
    bf = ml_dtypes.bfloat16
    in_maps = []
    for c in range(NCORES):
        b, g = c // GROUPS, c % GROUPS
        sl = slice(g * FS, (g + 1) * FS)
        in_maps.append({
            "xqT": np.ascontiguousarray(x_q[b].T.astype(bf)),
            "xkvT": np.ascontiguousarray(x_kv[b].T.astype(bf)),
            "wq": np.ascontiguousarray(Wq[:, sl].astype(bf)),
            "wk": np.ascontiguousarray(Wk[:, sl].astype(bf)),
            "wv": np.ascontiguousarray(Wv[:, sl].astype(bf)),
            "wo": np.ascontiguousarray(Wo[sl, :].astype(bf)),
            "bq": np.ascontiguousarray(bq_[sl]),
            "bk": np.ascontiguousarray(bk_[sl]),
            "bv": np.ascontiguousarray(bv_[sl]).reshape(1, FS),
            "bo": (bo_ if g == 0 else np.zeros_like(bo_)).reshape(1, D),
            "masks": mask,
        })
    return in_maps


def kernel(x_q, x_kv, Wq, bq, Wk, bk, Wv, bv, Wo, bo):
    x_q = np.asarray(x_q, dtype=np.float32)
    x_kv = np.asarray(x_kv, dtype=np.float32)
    if "nc" not in _CACHE:
        _CACHE["nc"] = _build()
    nc = _CACHE["nc"]
    in_maps = _shard_inputs(
        x_q, x_kv,
        np.asarray(Wq, np.float32), np.asarray(bq, np.float32),
        np.asarray(Wk, np.float32), np.asarray(bk, np.float32),
        np.asarray(Wv, np.float32), np.asarray(bv, np.float32),
        np.asarray(Wo, np.float32), np.asarray(bo, np.float32),
    )
    res = run_bass_kernel_spmd(nc, in_maps, core_ids=list(range(NCORES)))
    out = np.zeros((B, N, D), dtype=np.float32)
    for c in range(NCORES):
        out[c // GROUPS] += np.asarray(res.results[c]["out_p"], dtype=np.float32)
    return out


# revision 17
# speedup vs baseline: 1.7418x; 1.7418x over previous
"""Causal MHA (B=2, N=2048, D=1024, H=16) on 8 NeuronCores via Bass/Tile.

Sharding: core c = (b, g): b = c // 4 (batch), g = c % 4 (head group of 4
heads = 256 features). Each core computes its Q/K/V projections, causal
attention for its 4 heads, and a partial output projection (its 256 rows of
Wo). The host sums the 4 partials per batch ("unshard" of row-parallel TP).

Layout: activations are feature-major (features on SBUF partitions, sequence
on the free axis), so S^T = K Q^T tiles come out of the PE with k on
partitions and q free and exp() needs no reduction at all. The softmax
denominator falls out of the P@V matmul via a ones column appended to V; the
per-(head, q) normalization uses a reciprocal row broadcast across partitions
through a DRAM bounce.

v2 (throughput): all matmul operands are bf16 (host converts; 1 cycle/row at
any free size, vs f32r needing free>=256), which also halves DMA. Diagonal
causal blocks are trimmed to their unmasked q-range in S, exp and PV. During
attention the Activation engine runs ONLY the exp (projection epilogues and
masking/normalize run on the otherwise-idle Pool engine) since exp is the
pacing engine. PV lags S by one kt step, and ACT-independent PE work (the V
second-half pass, the previous slice's O-projection) is interleaved into the
attention loop so the PE never idles waiting for exp and keeps its 2.4 GHz
pstate (it drops to 1.2 GHz after idle gaps).
"""

import numpy as np
import ml_dtypes

import concourse.bass as bass
import concourse.bacc as bacc
import concourse.mybir as mybir
from concourse.tile import TileContext
from concourse.bass_utils import run_bass_kernel_spmd

F32 = mybir.dt.float32
BF16 = mybir.dt.bfloat16
AF = mybir.ActivationFunctionType

B, N, D, H, DH = 2, 2048, 1024, 16, 64
NCORES = 8
GROUPS = 4
HPC = H // GROUPS     # 4 heads per core
FS = HPC * DH         # 256
P = 128
NDT = N // 128        # 16
NSS = N // 512        # 4
DT = D // 128         # 8
FT = FS // 128        # 2
DH2 = DT // 2         # d-tiles per half

_CACHE = {}


def _build(repeat=1):
    nc = bacc.Bacc("TRN2", target_bir_lowering=False, debug=False)

    xqT = nc.dram_tensor("xqT", [D, N], BF16, kind="ExternalInput")
    xkvT = nc.dram_tensor("xkvT", [D, N], BF16, kind="ExternalInput")
    wq = nc.dram_tensor("wq", [D, FS], BF16, kind="ExternalInput")
    wk = nc.dram_tensor("wk", [D, FS], BF16, kind="ExternalInput")
    wv = nc.dram_tensor("wv", [D, FS], BF16, kind="ExternalInput")
    wo = nc.dram_tensor("wo", [FS, D], BF16, kind="ExternalInput")
    bq = nc.dram_tensor("bq", [FS], F32, kind="ExternalInput")
    bk = nc.dram_tensor("bk", [FS], F32, kind="ExternalInput")
    bv = nc.dram_tensor("bv", [1, FS], F32, kind="ExternalInput")
    bo = nc.dram_tensor("bo", [1, D], F32, kind="ExternalInput")
    masks = nc.dram_tensor("masks", [P, P], BF16, kind="ExternalInput")
    out = nc.dram_tensor("out_p", [N, D], BF16, kind="ExternalOutput")

    def bcast_rows(dram_ap, rows):
        # stride-0 partition AP: read DRAM row [1, F] as [rows, F]
        return bass.AP(
            tensor=dram_ap.tensor,
            offset=dram_ap.offset,
            ap=[[0, rows]] + [list(x) for x in dram_ap.ap[1:]],
        )

    with TileContext(nc) as tc:
        with (
            tc.tile_pool(name="const", bufs=1) as cp,
            tc.tile_pool(name="xt", bufs=1) as xp,
            tc.tile_pool(name="acts", bufs=1) as ap_,
            tc.tile_pool(name="ps", bufs=3, space="PSUM") as psp,
            tc.tile_pool(name="pt", bufs=3) as ptp,
            tc.tile_pool(name="small", bufs=4) as smp,
            tc.tile_pool(name="osb", bufs=3) as osp,
            tc.tile_pool(name="dsc", bufs=4, space="DRAM") as dsp,
        ):
            wo_sb = cp.tile([P, FT, D], BF16, tag="wo")
            bqk_sb = cp.tile([P, 2, 2], F32, tag="bqk")
            tri2_sb = cp.tile([P, 2, P], BF16, tag="mask")
            ones_f = cp.tile([P, HPC], BF16, tag="ones_f")
            bo_rep = cp.tile([P, D], F32, tag="bo_rep")
            bv_rep = cp.tile([P, FS], F32, tag="bv_rep")

            nc.sync.dma_start(out=wo_sb, in_=wo.ap().rearrange("(t p) f -> p t f", p=P))
            nc.sync.dma_start(out=bqk_sb[:, 0, :], in_=bk.ap().rearrange("(t p) -> p t", p=P))
            nc.sync.dma_start(out=bqk_sb[:, 1, :], in_=bq.ap().rearrange("(t p) -> p t", p=P))
            nc.sync.dma_start(out=tri2_sb[:, 0, :], in_=masks.ap())
            nc.sync.dma_start(out=tri2_sb[:, 1, :], in_=masks.ap())
            nc.sync.dma_start(out=bo_rep, in_=bcast_rows(bo.ap(), P))
            nc.sync.dma_start(out=bv_rep, in_=bcast_rows(bv.ap(), P))
            nc.vector.memset(ones_f, 1.0)

            kt_all = [ap_.tile([P, N], BF16, tag=f"kt{f}", name=f"kt{f}") for f in range(FT)]
            qt_all = [ap_.tile([P, N], BF16, tag=f"qt{f}", name=f"qt{f}") for f in range(FT)]
            v_sb = [ap_.tile([P, HPC, DH + 1], BF16, tag=f"v{st}", name=f"v{st}") for st in range(NDT)]
            ot_all = [ap_.tile([P, N], BF16, tag=f"ot{f}", name=f"ot{f}") for f in range(FT)]
            bv_rep_v = bv_rep.rearrange("p (h c) -> p h c", h=HPC)

            def emit_body():
                # ---- projections, streamed in two d-halves ----
                halves = []
                for half in range(2):
                    d0 = half * DH2
                    # weights go out on the ACT hwdge queue so the x tiles
                    # (SP queue) and the first K matmul aren't delayed
                    wk_sb = cp.tile([P, DH2, FS], BF16, tag="w", bufs=3, name="wk_h")
                    nc.scalar.dma_start(out=wk_sb, in_=wk.ap().rearrange("(t p) f -> p t f", p=P)[:, d0:d0 + DH2, :])
                    wv_sb = cp.tile([P, DH2, FS], BF16, tag="w", bufs=3, name="wv_h")
                    nc.scalar.dma_start(out=wv_sb, in_=wv.ap().rearrange("(t p) f -> p t f", p=P)[:, d0:d0 + DH2, :])
                    wq_sb = cp.tile([P, DH2, FS], BF16, tag="w", bufs=3, name="wq_h")
                    nc.scalar.dma_start(out=wq_sb, in_=wq.ap().rearrange("(t p) f -> p t f", p=P)[:, d0:d0 + DH2, :])
                    xkv_t, xq_t = [], []
                    for i in range(DH2):
                        d = d0 + i
                        t = xp.tile([P, N], BF16, tag=f"xkv{i}", name=f"xkv{i}")
                        nc.sync.dma_start(out=t, in_=xkvT.ap()[d * P:(d + 1) * P, :])
                        xkv_t.append(t)
                    for i in range(DH2):
                        d = d0 + i
                        t = xp.tile([P, N], BF16, tag=f"xq{i}", name=f"xq{i}")
                        nc.sync.dma_start(out=t, in_=xqT.ap()[d * P:(d + 1) * P, :])
                        xq_t.append(t)
                    halves.append((wk_sb, wv_sb, wq_sb, xkv_t, xq_t))

                def emit_kq(half, which, ft, ss):
                    wk_sb, wv_sb, wq_sb, xkv_t, xq_t = halves[half]
                    w_sb, x_t = (wk_sb, xkv_t) if which == 0 else (wq_sb, xq_t)
                    dst_all = kt_all if which == 0 else qt_all
                    ps = psp.tile([P, 512], F32, tag="ps", name="ps_kq")
                    for i in range(DH2):
                        nc.tensor.matmul(
                            ps,
                            w_sb[:, i, ft * P:(ft + 1) * P],
                            x_t[i][:, ss * 512:(ss + 1) * 512],
                            start=(i == 0),
                            stop=(i == DH2 - 1),
                        )
                    dst = dst_all[ft][:, ss * 512:(ss + 1) * 512]
                    if half == 0:
                        nc.vector.tensor_scalar_add(dst, ps, bqk_sb[:, which, ft:ft + 1])
                    else:
                        nc.vector.tensor_add(dst, dst, ps)

                def emit_v(half, st, pstag="ps"):
                    wk_sb, wv_sb, wq_sb, xkv_t, xq_t = halves[half]
                    psv = psp.tile([P, 512], F32, tag=pstag, bufs=(3 if pstag == "ps" else 1), name="ps_v")
                    for i in range(DH2):
                        nc.tensor.matmul(
                            psv[:, 0:FS],
                            xkv_t[i][:, st * P:(st + 1) * P],
                            wv_sb[:, i, :],
                            start=(i == 0),
                            stop=(i == DH2 - 1),
                        )
                    vdst = v_sb[st][:, :, 0:DH]
                    psv_v = psv[:, 0:FS].rearrange("p (h c) -> p h c", h=HPC)
                    if half == 0:
                        nc.vector.tensor_add(vdst, psv_v, bv_rep_v)
                        nc.vector.tensor_copy(v_sb[st][:, :, DH], ones_f)
                    else:
                        nc.vector.tensor_add(vdst, vdst, psv_v)

                # half 0: K, V, Q fully
                for ft in range(FT):
                    for ss in range(NSS):
                        emit_kq(0, 0, ft, ss)
                for st in range(NDT):
                    emit_v(0, st)
                for ft in range(FT):
                    for ss in range(NSS):
                        emit_kq(0, 1, ft, ss)
                # half 1: K, Q now; V interleaved into ss=0 attention below
                for ft in range(FT):
                    for ss in range(NSS):
                        emit_kq(1, 0, ft, ss)
                for ft in range(FT):
                    for ss in range(NSS):
                        emit_kq(1, 1, ft, ss)

                def oproj_units(ss_, tail=False):
                    for qt in range(4 * ss_, 4 * ss_ + 4):
                        o_sb = osp.tile([P, D], BF16, tag="osb", name="o_sb")
                        if tail:
                            # attention is done: st2's ps2 banks are free and
                            # double-buffered, so the tail pipeline never
                            # serializes on a single PSUM bank
                            def unit(qt=qt, o_sb=o_sb):
                                pst = psp.tile([P, 1024], F32, tag="ps2", bufs=2, name="ps_o2")
                                for os_ in range(2):
                                    for ft2 in range(FT):
                                        nc.tensor.matmul(
                                            pst[:, os_ * 512:(os_ + 1) * 512],
                                            ot_all[ft2][:, qt * P:(qt + 1) * P],
                                            wo_sb[:, ft2, os_ * 512:(os_ + 1) * 512],
                                            start=(ft2 == 0),
                                            stop=(ft2 == FT - 1),
                                        )
                                nc.vector.tensor_add(o_sb, pst, bo_rep)
                                nc.sync.dma_start(out=out.ap()[qt * P:(qt + 1) * P, :], in_=o_sb)
                            yield unit
                            continue
                        for os_ in range(2):
                            def unit(qt=qt, os_=os_, o_sb=o_sb):
                                ps_o = psp.tile([P, 512], F32, tag="pso", bufs=1, name="ps_o")
                                for ft2 in range(FT):
                                    nc.tensor.matmul(
                                        ps_o,
                                        ot_all[ft2][:, qt * P:(qt + 1) * P],
                                        wo_sb[:, ft2, os_ * 512:(os_ + 1) * 512],
                                        start=(ft2 == 0),
                                        stop=(ft2 == FT - 1),
                                    )
                                nc.vector.tensor_add(
                                    o_sb[:, os_ * 512:(os_ + 1) * 512],
                                    ps_o,
                                    bo_rep[:, os_ * 512:(os_ + 1) * 512],
                                )
                                if os_ == 1:
                                    nc.sync.dma_start(out=out.ap()[qt * P:(qt + 1) * P, :], in_=o_sb)
                            yield unit

                # ---- attention: per (ss, ft), kt loop with lag-1 PV and
                # PE filler units pulled between S and PV ----
                def attention_block(ss, ft, filler, quota):
                    # quota: how many filler units to pull across this block
                    n_kt = 4 * ss + 4
                    otp = [
                        psp.tile([P, 512], F32, tag="ps", name=f"ps_ot{hh}")
                        for hh in range(2)
                    ]
                    pulled = 0
                    pending = []  # PV lags S by 2 kt steps

                    def emit_pv(kt, ptt, dk):
                        for hh in range(2):
                            nc.tensor.matmul(
                                otp[hh][0:DH + 1, dk:512],
                                v_sb[kt][:, ft * 2 + hh, :],
                                ptt[:, hh * 512 + dk:(hh + 1) * 512],
                                start=(kt == 0),
                                stop=(kt == n_kt - 1),
                            )

                    for kt in range(n_kt):
                        dk = max(0, (kt - 4 * ss) * P)
                        st2 = psp.tile([P, 1024], F32, tag="ps2", bufs=2, name="ps_st2")
                        ptt = ptp.tile([P, 1024], BF16, tag="pt", name="ptt")
                        for hh in range(2):
                            nc.tensor.matmul(
                                st2[:, hh * 512 + dk:(hh + 1) * 512],
                                kt_all[ft][hh * 64:(hh + 1) * 64, kt * P:(kt + 1) * P],
                                qt_all[ft][hh * 64:(hh + 1) * 64, ss * 512 + dk:(ss + 1) * 512],
                                start=True, stop=True,
                            )
                        if len(pending) >= 2:
                            emit_pv(*pending.pop(0))
                        want = ((kt + 1) * quota) // n_kt
                        while pulled < want:
                            u = next(filler, None)
                            pulled += 1
                            if u is None:
                                break
                            u()
                        st2v = st2.rearrange("p (h q) -> p h q", h=2)
                        pttv = ptt.rearrange("p (h q) -> p h q", h=2)
                        nc.scalar.activation(pttv[:, :, dk:512], st2v[:, :, dk:512], AF.Exp, scale=0.125)
                        if kt >= 4 * ss:  # diagonal block: mask 128 cols past dk
                            nc.vector.tensor_mul(
                                pttv[:, :, dk:dk + P],
                                pttv[:, :, dk:dk + P],
                                tri2_sb,
                            )
                        pending.append((kt, ptt, dk))
                    for pv in pending:
                        emit_pv(*pv)

                    # normalization: reciprocal row, broadcast across
                    # partitions via a DRAM bounce (partition_broadcast ucode
                    # is broken in this environment)
                    rept = smp.tile([DH + 1, 1024], F32, tag="rep", bufs=2, name="rept")
                    recip = rept[DH:DH + 1, :]
                    with nc.allow_low_precision(reason="softmax reciprocal"):
                        nc.vector.reciprocal(recip[:, 0:512], otp[0][DH:DH + 1, :])
                        nc.vector.reciprocal(recip[:, 512:1024], otp[1][DH:DH + 1, :])
                    dscr = dsp.tile([1, 1024], F32, tag="dscr", name="dscr")
                    nc.sync.dma_start(out=dscr, in_=recip)
                    nc.sync.dma_start(out=rept[0:DH, :], in_=bcast_rows(dscr, DH))
                    for hh in range(2):
                        nc.vector.tensor_mul(
                            ot_all[ft][hh * 64:hh * 64 + DH, ss * 512:(ss + 1) * 512],
                            otp[hh][0:DH, :],
                            rept[0:DH, hh * 512:(hh + 1) * 512],
                        )

                # ss=0: fill with the 16 V-half1 tiles (uses the "pso" bank)
                v1 = iter([lambda st=st: emit_v(1, st, pstag="pso") for st in range(NDT)])
                attention_block(0, 0, v1, 8)
                attention_block(0, 1, v1, 8)
                # ss>=1: fill with the previous slice's O-projection
                for ss in range(1, NSS):
                    op = oproj_units(ss - 1)
                    attention_block(ss, 0, op, 4)
                    attention_block(ss, 1, op, 4)
                    for u in op:
                        u()
                for u in oproj_units(NSS - 1, tail=True):
                    u()

            if repeat == 1:
                emit_body()
            else:
                with tc.For_i(0, repeat, 1):
                    emit_body()

    nc.compile()
    return nc


def _shard_inputs(x_q, x_kv, Wq, bq_, Wk, bk_, Wv, bv_, Wo, bo_):
    pp_, ff = np.meshgrid(np.arange(P), np.arange(P), indexing="ij")
    mask = (ff >= pp_).astype(ml_dtypes.bfloat16)
    bf = ml_dtypes.bfloat16
    in_maps = []
    for c in range(NCORES):
        b, g = c // GROUPS, c % GROUPS
        sl = slice(g * FS, (g + 1) * FS)
        in_maps.append({
            "xqT": np.ascontiguousarray(x_q[b].T.astype(bf)),
            "xkvT": np.ascontiguousarray(x_kv[b].T.astype(bf)),
            "wq": np.ascontiguousarray(Wq[:, sl].astype(bf)),
            "wk": np.ascontiguousarray(Wk[:, sl].astype(bf)),
            "wv": np.ascontiguousarray(Wv[:, sl].astype(bf)),
            "wo": np.ascontiguousarray(Wo[sl, :].astype(bf)),
            "bq": np.ascontiguousarray(bq_[sl]),
            "bk": np.ascontiguousarray(bk_[sl]),
            "bv": np.ascontiguousarray(bv_[sl]).reshape(1, FS),
            "bo": (bo_ if g == 0 else np.zeros_like(bo_)).reshape(1, D),
            "masks": mask,
        })
    return in_maps


def kernel(x_q, x_kv, Wq, bq, Wk, bk, Wv, bv, Wo, bo):
    x_q = np.asarray(x_q, dtype=np.float32)
    x_kv = np.asarray(x_kv, dtype=np.float32)
    if "nc" not in _CACHE:
        _CACHE["nc"] = _build()
    nc = _CACHE["nc"]
    in_maps = _shard_inputs(
        x_q, x_kv,
        np.asarray(Wq, np.float32), np.asarray(bq, np.float32),
        np.asarray(Wk, np.float32), np.asarray(bk, np.float32),
        np.asarray(Wv, np.float32), np.asarray(bv, np.float32),
        np.asarray(Wo, np.float32), np.asarray(bo, np.float32),
    )
    res = run_bass_kernel_spmd(nc, in_maps, core_ids=list(range(NCORES)))
    out = np.zeros((B, N, D), dtype=np.float32)
    for c in range(NCORES):
        out[c // GROUPS] += np.asarray(res.results[c]["out_p"], dtype=np.float32)
    return out


# revision 20
# speedup vs baseline: 3.1736x; 1.8220x over previous
"""Causal MHA (B=2, N=2048, D=1024, H=16) on 8 NeuronCores via Bass/Tile.

Sharding: core c = (b, g): b = c // 4 (batch), g = c % 4 (head group of 4
heads = 256 features). Each core computes its Q/K/V projections, causal
attention for its 4 heads, and a partial output projection (its 256 rows of
Wo). The host sums the 4 partials per batch ("unshard" of row-parallel TP).

Layout: activations are feature-major (features on SBUF partitions, sequence
on the free axis), so S^T = K Q^T tiles come out of the PE with k on
partitions and q free and exp() needs no reduction at all. The softmax
denominator falls out of the P@V matmul via a ones column appended to V; the
per-(head, q) normalization uses a reciprocal row broadcast across partitions
through a DRAM bounce.

All matmuls are float32r and self-load weights: measured on this HW, a bf16
matmul pays ~174ns extra for its separate non-overlapped Ldweights, which is
strictly worse. f32r runs 1 cycle/row at moving free >= 256, so causal
diagonal blocks are trimmed only down to 256 columns (the exp is trimmed to
the exact unmasked range; a small memset re-zeroes the 128-column gap when
the mask offset is 384). GPSIMD/Pool compute ops are never used (measured
~5.5us each on HW). During attention the Activation engine runs ONLY the exp
(it is the pacing engine: ~1.03us per kt step vs the PE's ~0.85us); all
projection/normalize epilogues run on the DVE. PV lags S by two kt steps so
the S->exp->mask->PV chain never stalls the PE, and ACT-independent PE work
(the V second-half pass during ss=0, the previous slice's O-projection
after) is interleaved into the attention loop at kt granularity so the PE
keeps its 2.4 GHz pstate (it drops to 1.2 GHz after idle gaps).
"""

import numpy as np

import concourse.bass as bass
import concourse.bacc as bacc
import concourse.mybir as mybir
from concourse.tile import TileContext
from concourse.bass_utils import run_bass_kernel_spmd

F32 = mybir.dt.float32
F32R = mybir.dt.float32r
AF = mybir.ActivationFunctionType

B, N, D, H, DH = 2, 2048, 1024, 16, 64
NCORES = 8
GROUPS = 4
HPC = H // GROUPS     # 4 heads per core
FS = HPC * DH         # 256
P = 128
NDT = N // 128        # 16
NSS = N // 512        # 4
DT = D // 128         # 8
FT = FS // 128        # 2
DH2 = DT // 2         # d-tiles per half

_CACHE = {}


def _build(repeat=1):
    nc = bacc.Bacc("TRN2", target_bir_lowering=False, debug=False)

    xqT = nc.dram_tensor("xqT", [D, N], F32R, kind="ExternalInput")
    xkvT = nc.dram_tensor("xkvT", [D, N], F32R, kind="ExternalInput")
    wq = nc.dram_tensor("wq", [D, FS], F32R, kind="ExternalInput")
    wk = nc.dram_tensor("wk", [D, FS], F32R, kind="ExternalInput")
    wv = nc.dram_tensor("wv", [D, FS], F32R, kind="ExternalInput")
    wo = nc.dram_tensor("wo", [FS, D], F32R, kind="ExternalInput")
    bq = nc.dram_tensor("bq", [FS], F32, kind="ExternalInput")
    bk = nc.dram_tensor("bk", [FS], F32, kind="ExternalInput")
    bv = nc.dram_tensor("bv", [1, FS], F32, kind="ExternalInput")
    bo = nc.dram_tensor("bo", [1, D], F32, kind="ExternalInput")
    masks = nc.dram_tensor("masks", [P, P], F32R, kind="ExternalInput")
    out = nc.dram_tensor("out_p", [N, D], F32, kind="ExternalOutput")

    def bcast_rows(src, rows):
        # stride-0 partition AP: read a [1, F] row as [rows, F]
        return bass.AP(
            tensor=src.tensor,
            offset=src.offset,
            ap=[[0, rows]] + [list(x) for x in src.ap[1:]],
        )

    with TileContext(nc) as tc:
        with (
            tc.tile_pool(name="const", bufs=1) as cp,
            tc.tile_pool(name="xt", bufs=1) as xp,
            tc.tile_pool(name="acts", bufs=1) as ap_,
            tc.tile_pool(name="ps", bufs=3, space="PSUM") as psp,
            tc.tile_pool(name="pt", bufs=3) as ptp,
            tc.tile_pool(name="small", bufs=4) as smp,
            tc.tile_pool(name="osb", bufs=3) as osp,
            tc.tile_pool(name="dsc", bufs=4, space="DRAM") as dsp,
        ):
            wo_sb = cp.tile([P, FT, D], F32R, tag="wo")
            bqk_sb = cp.tile([P, 2, 2], F32, tag="bqk")
            tri2_sb = cp.tile([P, 2, P], F32R, tag="mask")
            ones_f = cp.tile([P, HPC], F32, tag="ones_f")
            bo_rep = cp.tile([P, D], F32, tag="bo_rep")
            bv_rep = cp.tile([P, FS], F32, tag="bv_rep")

            nc.sync.dma_start(out=wo_sb, in_=wo.ap().rearrange("(t p) f -> p t f", p=P))
            nc.sync.dma_start(out=bqk_sb[:, 0, :], in_=bk.ap().rearrange("(t p) -> p t", p=P))
            nc.sync.dma_start(out=bqk_sb[:, 1, :], in_=bq.ap().rearrange("(t p) -> p t", p=P))
            nc.sync.dma_start(out=tri2_sb[:, 0, :], in_=masks.ap())
            nc.sync.dma_start(out=tri2_sb[:, 1, :], in_=masks.ap())
            nc.sync.dma_start(out=bo_rep, in_=bcast_rows(bo.ap(), P))
            nc.sync.dma_start(out=bv_rep, in_=bcast_rows(bv.ap(), P))
            nc.vector.memset(ones_f, 1.0)

            kt_all = [ap_.tile([P, N], F32R, tag=f"kt{f}", name=f"kt{f}") for f in range(FT)]
            qt_all = [ap_.tile([P, N], F32R, tag=f"qt{f}", name=f"qt{f}") for f in range(FT)]
            v_sb = [ap_.tile([P, HPC, DH + 1], F32R, tag=f"v{st}", name=f"v{st}") for st in range(NDT)]
            ot_all = [ap_.tile([P, N], F32R, tag=f"ot{f}", name=f"ot{f}") for f in range(FT)]
            bv_rep_v = bv_rep.rearrange("p (h c) -> p h c", h=HPC)

            def emit_body():
                # ---- projections, streamed in two d-halves ----
                halves = []
                for half in range(2):
                    d0 = half * DH2
                    # weights go out on the ACT hwdge queue so the x tiles
                    # (SP queue) and the first K matmul aren't delayed
                    wk_sb = cp.tile([P, DH2, FS], F32R, tag="w", bufs=3, name="wk_h")
                    nc.scalar.dma_start(out=wk_sb, in_=wk.ap().rearrange("(t p) f -> p t f", p=P)[:, d0:d0 + DH2, :])
                    wv_sb = cp.tile([P, DH2, FS], F32R, tag="w", bufs=3, name="wv_h")
                    nc.scalar.dma_start(out=wv_sb, in_=wv.ap().rearrange("(t p) f -> p t f", p=P)[:, d0:d0 + DH2, :])
                    wq_sb = cp.tile([P, DH2, FS], F32R, tag="w", bufs=3, name="wq_h")
                    nc.scalar.dma_start(out=wq_sb, in_=wq.ap().rearrange("(t p) f -> p t f", p=P)[:, d0:d0 + DH2, :])
                    xkv_t, xq_t = [], []
                    for i in range(DH2):
                        d = d0 + i
                        t = xp.tile([P, N], F32R, tag=f"xkv{i}", name=f"xkv{i}")
                        nc.sync.dma_start(out=t, in_=xkvT.ap()[d * P:(d + 1) * P, :])
                        xkv_t.append(t)
                    for i in range(DH2):
                        d = d0 + i
                        t = xp.tile([P, N], F32R, tag=f"xq{i}", name=f"xq{i}")
                        nc.sync.dma_start(out=t, in_=xqT.ap()[d * P:(d + 1) * P, :])
                        xq_t.append(t)
                    halves.append((wk_sb, wv_sb, wq_sb, xkv_t, xq_t))

                def emit_kq(half, which, ft, ss):
                    wk_sb, wv_sb, wq_sb, xkv_t, xq_t = halves[half]
                    w_sb, x_t = (wk_sb, xkv_t) if which == 0 else (wq_sb, xq_t)
                    dst_all = kt_all if which == 0 else qt_all
                    ps = psp.tile([P, 512], F32, tag="ps", name="ps_kq")
                    for i in range(DH2):
                        nc.tensor.matmul(
                            ps,
                            w_sb[:, i, ft * P:(ft + 1) * P],
                            x_t[i][:, ss * 512:(ss + 1) * 512],
                            start=(i == 0),
                            stop=(i == DH2 - 1),
                        )
                    dst = dst_all[ft][:, ss * 512:(ss + 1) * 512]
                    if half == 0:
                        nc.vector.tensor_scalar_add(dst, ps, bqk_sb[:, which, ft:ft + 1])
                    else:
                        nc.vector.tensor_add(dst, dst, ps)

                def emit_v(half, st, pstag="ps"):
                    wk_sb, wv_sb, wq_sb, xkv_t, xq_t = halves[half]
                    psv = psp.tile([P, 512], F32, tag=pstag, bufs=(3 if pstag == "ps" else 1), name="ps_v")
                    for i in range(DH2):
                        nc.tensor.matmul(
                            psv[:, 0:FS],
                            xkv_t[i][:, st * P:(st + 1) * P],
                            wv_sb[:, i, :],
                            start=(i == 0),
                            stop=(i == DH2 - 1),
                        )
                    vdst = v_sb[st][:, :, 0:DH]
                    psv_v = psv[:, 0:FS].rearrange("p (h c) -> p h c", h=HPC)
                    if half == 0:
                        nc.vector.tensor_add(vdst, psv_v, bv_rep_v)
                        nc.vector.tensor_copy(v_sb[st][:, :, DH], ones_f)
                    else:
                        nc.vector.tensor_add(vdst, vdst, psv_v)

                # half 0: K, V, Q fully
                for ft in range(FT):
                    for ss in range(NSS):
                        emit_kq(0, 0, ft, ss)
                for st in range(NDT):
                    emit_v(0, st)
                for ft in range(FT):
                    for ss in range(NSS):
                        emit_kq(0, 1, ft, ss)
                # half 1: K, Q now; V interleaved into ss=0 attention below
                for ft in range(FT):
                    for ss in range(NSS):
                        emit_kq(1, 0, ft, ss)
                for ft in range(FT):
                    for ss in range(NSS):
                        emit_kq(1, 1, ft, ss)

                def oproj_units(ss_, tail=False):
                    for qt in range(4 * ss_, 4 * ss_ + 4):
                        o_sb = osp.tile([P, D], F32, tag="osb", name="o_sb")
                        if tail:
                            # attention is done: st2's ps2 banks are free and
                            # double-buffered, so the tail pipeline never
                            # serializes on a single PSUM bank
                            def unit(qt=qt, o_sb=o_sb):
                                pst = psp.tile([P, 1024], F32, tag="ps2", bufs=2, name="ps_o2")
                                for os_ in range(2):
                                    for ft2 in range(FT):
                                        nc.tensor.matmul(
                                            pst[:, os_ * 512:(os_ + 1) * 512],
                                            ot_all[ft2][:, qt * P:(qt + 1) * P],
                                            wo_sb[:, ft2, os_ * 512:(os_ + 1) * 512],
                                            start=(ft2 == 0),
                                            stop=(ft2 == FT - 1),
                                        )
                                nc.vector.tensor_add(o_sb, pst, bo_rep)
                                nc.sync.dma_start(out=out.ap()[qt * P:(qt + 1) * P, :], in_=o_sb)
                            yield unit
                            continue
                        for os_ in range(2):
                            def unit(qt=qt, os_=os_, o_sb=o_sb):
                                ps_o = psp.tile([P, 512], F32, tag="pso", bufs=1, name="ps_o")
                                for ft2 in range(FT):
                                    nc.tensor.matmul(
                                        ps_o,
                                        ot_all[ft2][:, qt * P:(qt + 1) * P],
                                        wo_sb[:, ft2, os_ * 512:(os_ + 1) * 512],
                                        start=(ft2 == 0),
                                        stop=(ft2 == FT - 1),
                                    )
                                nc.vector.tensor_add(
                                    o_sb[:, os_ * 512:(os_ + 1) * 512],
                                    ps_o,
                                    bo_rep[:, os_ * 512:(os_ + 1) * 512],
                                )
                                if os_ == 1:
                                    nc.sync.dma_start(out=out.ap()[qt * P:(qt + 1) * P, :], in_=o_sb)
                            yield unit

                # ---- attention: per (ss, ft), kt loop with lag-2 PV and
                # PE filler units pulled between S and PV ----
                def attention_block(ss, ft, filler, quota):
                    n_kt = 4 * ss + 4
                    otp = [
                        psp.tile([P, 512], F32, tag="ps", name=f"ps_ot{hh}")
                        for hh in range(2)
                    ]
                    pulled = 0
                    pending = []  # PV lags S by 2 kt steps

                    def emit_pv(kt, ptt, mm_dk):
                        for hh in range(2):
                            nc.tensor.matmul(
                                otp[hh][0:DH + 1, mm_dk:512],
                                v_sb[kt][:, ft * 2 + hh, :],
                                ptt[:, hh * 512 + mm_dk:(hh + 1) * 512],
                                start=(kt == 0),
                                stop=(kt == n_kt - 1),
                            )

                    for kt in range(n_kt):
                        dk = max(0, (kt - 4 * ss) * P)
                        # f32r needs moving free >= 256 for full rate: don't
                        # trim the matmuls below 256 columns
                        mm_dk = min(dk, 256)
                        st2 = psp.tile([P, 1024], F32, tag="ps2", bufs=2, name="ps_st2")
                        ptt = ptp.tile([P, 1024], F32R, tag="pt", name="ptt")
                        for hh in range(2):
                            nc.tensor.matmul(
                                st2[:, hh * 512 + mm_dk:(hh + 1) * 512],
                                kt_all[ft][hh * 64:(hh + 1) * 64, kt * P:(kt + 1) * P],
                                qt_all[ft][hh * 64:(hh + 1) * 64, ss * 512 + mm_dk:(ss + 1) * 512],
                                start=True, stop=True,
                            )
                        if len(pending) >= 2:
                            emit_pv(*pending.pop(0))
                        want = ((kt + 1) * quota) // n_kt
                        while pulled < want:
                            u = next(filler, None)
                            pulled += 1
                            if u is None:
                                break
                            u()
                        st2v = st2.rearrange("p (h q) -> p h q", h=2)
                        pttv = ptt.rearrange("p (h q) -> p h q", h=2)
                        nc.scalar.activation(pttv[:, :, dk:512], st2v[:, :, dk:512], AF.Exp, scale=0.125)
                        if dk > mm_dk:
                            # PV reads [mm_dk:512]; zero the fully-masked gap
                            # the exp didn't write (memset on f32r is invalid
                            # ISA; scalar-mul by 0 works)
                            nc.vector.tensor_scalar_mul(
                                pttv[:, :, mm_dk:dk],
                                pttv[:, :, mm_dk:dk],
                                0.0,
                            )
                        if kt >= 4 * ss:  # diagonal block: mask 128 cols past dk
                            nc.vector.tensor_mul(
                                pttv[:, :, dk:dk + P],
                                pttv[:, :, dk:dk + P],
                                tri2_sb,
                            )
                        pending.append((kt, ptt, mm_dk))
                    for pv in pending:
                        emit_pv(*pv)

                    # normalization: reciprocal row, broadcast across
                    # partitions via a DRAM bounce
                    rept = smp.tile([DH + 1, 1024], F32, tag="rep", bufs=2, name="rept")
                    recip = rept[DH:DH + 1, :]
                    with nc.allow_low_precision(reason="softmax reciprocal"):
                        nc.vector.reciprocal(recip[:, 0:512], otp[0][DH:DH + 1, :])
                        nc.vector.reciprocal(recip[:, 512:1024], otp[1][DH:DH + 1, :])
                    dscr = dsp.tile([1, 1024], F32, tag="dscr", name="dscr")
                    nc.sync.dma_start(out=dscr, in_=recip)
                    nc.sync.dma_start(out=rept[0:DH, :], in_=bcast_rows(dscr, DH))
                    for hh in range(2):
                        nc.vector.tensor_mul(
                            ot_all[ft][hh * 64:hh * 64 + DH, ss * 512:(ss + 1) * 512],
                            otp[hh][0:DH, :],
                            rept[0:DH, hh * 512:(hh + 1) * 512],
                        )

                # ss=0: fill with the 16 V-half1 tiles (uses the "pso" bank)
                v1 = iter([lambda st=st: emit_v(1, st, pstag="pso") for st in range(NDT)])
                attention_block(0, 0, v1, 8)
                attention_block(0, 1, v1, 8)
                # ss>=1: fill with the previous slice's O-projection
                for ss in range(1, NSS):
                    op = oproj_units(ss - 1)
                    attention_block(ss, 0, op, 4)
                    attention_block(ss, 1, op, 4)
                    for u in op:
                        u()
                for u in oproj_units(NSS - 1, tail=True):
                    u()

            if repeat == 1:
                emit_body()
            else:
                with tc.For_i(0, repeat, 1):
                    emit_body()

    nc.compile()
    return nc


def _shard_inputs(x_q, x_kv, Wq, bq_, Wk, bk_, Wv, bv_, Wo, bo_):
    pp_, ff = np.meshgrid(np.arange(P), np.arange(P), indexing="ij")
    mask = (ff >= pp_).astype(np.float32)
    in_maps = []
    for c in range(NCORES):
        b, g = c // GROUPS, c % GROUPS
        sl = slice(g * FS, (g + 1) * FS)
        in_maps.append({
            "xqT": np.ascontiguousarray(x_q[b].T),
            "xkvT": np.ascontiguousarray(x_kv[b].T),
            "wq": np.ascontiguousarray(Wq[:, sl]),
            "wk": np.ascontiguousarray(Wk[:, sl]),
            "wv": np.ascontiguousarray(Wv[:, sl]),
            "wo": np.ascontiguousarray(Wo[sl, :]),
            "bq": np.ascontiguousarray(bq_[sl]),
            "bk": np.ascontiguousarray(bk_[sl]),
            "bv": np.ascontiguousarray(bv_[sl]).reshape(1, FS),
            "bo": (bo_ if g == 0 else np.zeros_like(bo_)).reshape(1, D),
            "masks": mask,
        })
    return in_maps


def kernel(x_q, x_kv, Wq, bq, Wk, bk, Wv, bv, Wo, bo):
    x_q = np.asarray(x_q, dtype=np.float32)
    x_kv = np.asarray(x_kv, dtype=np.float32)
    if "nc" not in _CACHE:
        _CACHE["nc"] = _build()
    nc = _CACHE["nc"]
    in_maps = _shard_inputs(
        x_q, x_kv,
        np.asarray(Wq, np.float32), np.asarray(bq, np.float32),
        np.asarray(Wk, np.float32), np.asarray(bk, np.float32),
        np.asarray(Wv, np.float32), np.asarray(bv, np.float32),
        np.asarray(Wo, np.float32), np.asarray(bo, np.float32),
    )
    res = run_bass_kernel_spmd(nc, in_maps, core_ids=list(range(NCORES)))
    out = np.zeros((B, N, D), dtype=np.float32)
    for c in range(NCORES):
        out[c // GROUPS] += np.asarray(res.results[c]["out_p"], dtype=np.float32)
    return out
